# revision 3
# baseline (speedup 1.0000x reference)
"""Multi-head attention (16 heads, d_model=1024, bs=2, qlen=2048) on 8 trn2 cores.

Sharding: core c -> batch b = c//4, head-group r = c%4 (heads 4r..4r+3, i.e.
dims 256r..256r+256 of the head axis).  Each core projects q/k/v only for its
own 4 heads (Megatron column split), runs scores/softmax/AV for those heads,
then an AllGather of the per-core context slice within each batch group of 4
cores provides the full 1024-dim context for the row-split output projection
(each core computes its own 256 output columns; no reduction needed).

Numerics: bf16 matmul operands, fp32 PSUM accumulation, softmax in fp32 on the
scalar engine.  Scores are computed transposed (k on partitions) so the mask
is a per-partition bias and the softmax denominator comes from a ones-matmul
that lands broadcast across partitions (elementwise normalize, no
cross-partition ops).  1/sqrt(d) and q_b are folded into q_w/q_b on the host;
v_b is deferred past the softmax (rows of P/sum sum to 1) and added to the
normalized context.
"""

import functools
import os
import sys

import numpy as np

for _p in ("/opt/trn_rl_repo", "/root/.axon_site/_ro/trn_rl_repo"):
    if os.path.isdir(_p) and _p not in sys.path:
        sys.path.append(_p)

import ml_dtypes

from concourse import bacc, bass, mybir, tile
from concourse.bass_utils import run_bass_kernel_spmd

BF16 = ml_dtypes.bfloat16
FP32 = mybir.dt.float32
BF16_DT = mybir.dt.bfloat16

N_CORES = 8
BS = 2
L = 2048  # sequence length
D = 1024  # model dim
DH = 64  # head dim
OWN = 256  # head dims per core (4 heads)
KC_D = 8  # 1024 / 128 contraction chunks for projections
NT = 4  # 2048 / 512 token tiles
KT = 16  # 2048 / 128 key-token chunks
ACT_GRP = 2  # k-chunks per exp() activation op

# exp-time info for the last run (filled when KERNEL_TRACE=1)
LAST_EXEC_NS = None
LAST_RESULTS = None


def _build_nc(apply_mask: bool):
    nc = bacc.Bacc(None, num_devices=N_CORES)

    xT = nc.dram_tensor("xT", [D, L], BF16_DT, kind="ExternalInput")
    wq = nc.dram_tensor("wq", [D, OWN], BF16_DT, kind="ExternalInput")
    wk = nc.dram_tensor("wk", [D, OWN], BF16_DT, kind="ExternalInput")
    wv = nc.dram_tensor("wv", [D, OWN], BF16_DT, kind="ExternalInput")
    wo = nc.dram_tensor("wo", [D, OWN], BF16_DT, kind="ExternalInput")
    qb2 = nc.dram_tensor("qb2", [128, 2], FP32, kind="ExternalInput")
    kb2 = nc.dram_tensor("kb2", [128, 2], FP32, kind="ExternalInput")
    vb2 = nc.dram_tensor("vb2", [128, 2], FP32, kind="ExternalInput")
    ob2 = nc.dram_tensor("ob2", [128, 2], FP32, kind="ExternalInput")
    mask01 = nc.dram_tensor("mask01", [128, KT], FP32, kind="ExternalInput")
    outT = nc.dram_tensor("outT", [OWN, L], FP32, kind="ExternalOutput")

    Exp = mybir.ActivationFunctionType.Exp

    with tile.TileContext(nc) as tc:
        with (
            tc.tile_pool(name="const", bufs=1) as const,
            tc.tile_pool(name="work", bufs=2) as work,
            tc.tile_pool(name="ps", bufs=1, space="PSUM") as ps,
            tc.tile_pool(name="dram", bufs=1, space="DRAM") as dram,
        ):
            # ---- stage inputs into SBUF ----
            x_sb = []
            for i in range(KC_D):
                t = const.tile([128, L], BF16_DT, tag=f"x{i}", name=f"x_sb{i}")
                nc.sync.dma_start(t, xT[i * 128 : (i + 1) * 128, :])
                x_sb.append(t)

            def load_w(dram_t, nm):
                tiles = []
                for i in range(KC_D):
                    t = const.tile([128, OWN], BF16_DT, tag=f"{nm}{i}", name=f"{nm}_sb{i}")
                    nc.sync.dma_start(t, dram_t[i * 128 : (i + 1) * 128, :])
                    tiles.append(t)
                return tiles

            wq_sb = load_w(wq, "wq")
            wk_sb = load_w(wk, "wk")
            wv_sb = load_w(wv, "wv")
            wo_sb = load_w(wo, "wo")

            def load_small(dram_t, nm, cols):
                t = const.tile([128, cols], FP32, tag=nm, name=f"{nm}_sb")
                nc.sync.dma_start(t, dram_t[:, :])
                return t

            qb_sb = load_small(qb2, "qb", 2)
            kb_sb = load_small(kb2, "kb", 2)
            vb_sb = load_small(vb2, "vb", 2)
            ob_sb = load_small(ob2, "ob", 2)
            mask_sb = load_small(mask01, "mask", KT) if apply_mask else None

            ones_sb = const.tile([128, DH], BF16_DT, tag="ones", name="ones_sb")
            nc.vector.memset(ones_sb, 1.0)

            # ---- q/k projections (transposed: own-dim on partitions) ----
            # qT = (Wq_own @ x.T): lhsT = wq chunk [128k, 128m], rhs = x.T chunk
            qT_sb = [
                const.tile([128, L], BF16_DT, tag=f"qT{p}", name=f"qT_sb{p}")
                for p in range(2)
            ]
            kT_sb = [
                const.tile([128, L], BF16_DT, tag=f"kT{p}", name=f"kT_sb{p}")
                for p in range(2)
            ]
            for w_sb, b_sb, dst in ((wq_sb, qb_sb, qT_sb), (wk_sb, kb_sb, kT_sb)):
                for m in range(2):
                    for n in range(NT):
                        pp = ps.tile(
                            [128, 512], FP32, tag=f"s{(m * NT + n) % 2}", name="proj_ps"
                        )
                        for kc in range(KC_D):
                            nc.tensor.matmul(
                                pp,
                                lhsT=w_sb[kc][:, m * 128 : (m + 1) * 128],
                                rhs=x_sb[kc][:, n * 512 : (n + 1) * 512],
                                start=(kc == 0),
                                stop=(kc == KC_D - 1),
                            )
                        nc.vector.tensor_scalar_add(
                            dst[m][:, n * 512 : (n + 1) * 512], pp, b_sb[:, m : m + 1]
                        )

            # ---- v projection (untransposed: tokens on partitions; no bias) ----
            v_sb = [
                const.tile([128, OWN], BF16_DT, tag=f"v{t}", name=f"v_sb{t}")
                for t in range(KT)
            ]
            for t in range(KT):
                pv = ps.tile([128, OWN], FP32, tag=f"s{t % 2}", name="v_ps")
                for kc in range(KC_D):
                    nc.tensor.matmul(
                        pv,
                        lhsT=x_sb[kc][:, t * 128 : (t + 1) * 128],
                        rhs=wv_sb[kc],
                        start=(kc == 0),
                        stop=(kc == KC_D - 1),
                    )
                nc.vector.tensor_copy(v_sb[t], pv)

            # ---- attention (2 head pairs x 4 q-tiles) ----
            ctx_sb = [
                const.tile([128, L], BF16_DT, tag=f"ctx{p}", name=f"ctx_sb{p}")
                for p in range(2)
            ]
            n_grp = KT // ACT_GRP
            for p in range(2):
                for qt in range(NT):
                    ctx_ps = ps.tile([128, 512], FP32, tag="ctx", bufs=2, name="ctx_ps")
                    sum_ps = ps.tile([128, 512], FP32, tag="sums", bufs=2, name="sum_ps")
                    q0 = qT_sb[p][0:64, qt * 512 : (qt + 1) * 512]
                    q1 = qT_sb[p][64:128, qt * 512 : (qt + 1) * 512]
                    for g in range(n_grp):
                        s0 = ps.tile([128, 512 * ACT_GRP], FP32, tag="s0", name="s0")
                        s1 = ps.tile([128, 512 * ACT_GRP], FP32, tag="s1", name="s1")
                        for j in range(ACT_GRP):
                            kc = g * ACT_GRP + j
                            # scores^T = K @ Q^T, two heads row-packed (K=64 each)
                            nc.tensor.matmul(
                                s0[:, j * 512 : (j + 1) * 512],
                                lhsT=kT_sb[p][0:64, kc * 128 : (kc + 1) * 128],
                                rhs=q0,
                            )
                            nc.tensor.matmul(
                                s1[:, j * 512 : (j + 1) * 512],
                                lhsT=kT_sb[p][64:128, kc * 128 : (kc + 1) * 128],
                                rhs=q1,
                            )
                        pr0 = work.tile(
                            [128, 512 * ACT_GRP], BF16_DT, tag="pr0", name="pr0"
                        )
                        pr1 = work.tile(
                            [128, 512 * ACT_GRP], BF16_DT, tag="pr1", name="pr1"
                        )
                        if apply_mask:
                            # exp then zero out masked keys (per-partition mask)
                            e0 = work.tile(
                                [128, 512 * ACT_GRP], FP32, tag="e0", name="e0"
                            )
                            e1 = work.tile(
                                [128, 512 * ACT_GRP], FP32, tag="e1", name="e1"
                            )
                            nc.scalar.activation(e0, s0, Exp)
                            nc.scalar.activation(e1, s1, Exp)
                            for j in range(ACT_GRP):
                                kc = g * ACT_GRP + j
                                sl = slice(j * 512, (j + 1) * 512)
                                nc.vector.tensor_scalar_mul(
                                    pr0[:, sl], e0[:, sl], mask_sb[:, kc : kc + 1]
                                )
                                nc.vector.tensor_scalar_mul(
                                    pr1[:, sl], e1[:, sl], mask_sb[:, kc : kc + 1]
                                )
                        else:
                            nc.scalar.activation(pr0, s0, Exp)
                            nc.scalar.activation(pr1, s1, Exp)
                        for j in range(ACT_GRP):
                            kc = g * ACT_GRP + j
                            sl = slice(j * 512, (j + 1) * 512)
                            st = kc == 0
                            sp = kc == KT - 1
                            # context^T accumulation, two heads column-packed
                            nc.tensor.matmul(
                                ctx_ps[0:64, :],
                                lhsT=v_sb[kc][:, p * 128 : p * 128 + 64],
                                rhs=pr0[:, sl],
                                start=st,
                                stop=sp,
                            )
                            nc.tensor.matmul(
                                ctx_ps[64:128, :],
                                lhsT=v_sb[kc][:, p * 128 + 64 : p * 128 + 128],
                                rhs=pr1[:, sl],
                                start=st,
                                stop=sp,
                            )
                            # softmax denominators, broadcast across partitions
                            nc.tensor.matmul(
                                sum_ps[0:64, :],
                                lhsT=ones_sb,
                                rhs=pr0[:, sl],
                                start=st,
                                stop=sp,
                            )
                            nc.tensor.matmul(
                                sum_ps[64:128, :],
                                lhsT=ones_sb,
                                rhs=pr1[:, sl],
                                start=st,
                                stop=sp,
                            )
                    recip = work.tile([128, 512], FP32, tag="recip", name="recip")
                    nc.vector.reciprocal(recip, sum_ps)
                    tmp = work.tile([128, 512], FP32, tag="tmp", name="tmp")
                    nc.vector.tensor_mul(tmp, ctx_ps, recip)
                    nc.vector.tensor_scalar_add(
                        ctx_sb[p][:, qt * 512 : (qt + 1) * 512],
                        tmp,
                        vb_sb[:, p : p + 1],
                    )

            # ---- AllGather context within each 4-core batch group ----
            ag_in = dram.tile([OWN, L], BF16_DT, name="ag_in")
            ag_out = dram.tile([D, L], BF16_DT, name="ag_out")
            nc.sync.dma_start(ag_in[0:128, :], ctx_sb[0])
            nc.sync.dma_start(ag_in[128:256, :], ctx_sb[1])
            nc.gpsimd.collective_compute(
                "AllGather",
                mybir.AluOpType.bypass,
                replica_groups=[[0, 1, 2, 3], [4, 5, 6, 7]],
                ins=[ag_in.opt()],
                outs=[ag_out.opt()],
            )

            # ---- output projection (row split; own 256 output columns) ----
            cf = []
            for kc in range(KC_D):
                t = const.tile([128, L], BF16_DT, tag=f"cf{kc}", name=f"cf{kc}")
                nc.sync.dma_start(t, ag_out[kc * 128 : (kc + 1) * 128, :])
                cf.append(t)
            for m in range(2):
                for n in range(NT):
                    po = ps.tile(
                        [128, 512], FP32, tag=f"s{(m * NT + n) % 2}", name="o_ps"
                    )
                    for kc in range(KC_D):
                        nc.tensor.matmul(
                            po,
                            lhsT=wo_sb[kc][:, m * 128 : (m + 1) * 128],
                            rhs=cf[kc][:, n * 512 : (n + 1) * 512],
                            start=(kc == 0),
                            stop=(kc == KC_D - 1),
                        )
                    osb = work.tile([128, 512], FP32, tag="osb", name="osb")
                    nc.vector.tensor_scalar_add(osb, po, ob_sb[:, m : m + 1])
                    nc.sync.dma_start(
                        outT[m * 128 : (m + 1) * 128, n * 512 : (n + 1) * 512], osb
                    )

    nc.finalize()
    return nc


@functools.lru_cache(maxsize=2)
def _built(apply_mask: bool):
    return _build_nc(apply_mask)


def kernel(input, mask, q_w, q_b, k_w, k_b, v_w, v_b, o_w, o_b):
    global LAST_EXEC_NS, LAST_RESULTS
    input = np.asarray(input, dtype=np.float32)
    mask = np.asarray(mask)
    apply_mask = not bool(np.all(mask != 0))
    nc = _built(apply_mask)

    qw = (np.asarray(q_w, np.float32) / 8.0).astype(BF16)
    kw = np.asarray(k_w, np.float32).astype(BF16)
    vw = np.asarray(v_w, np.float32).astype(BF16)
    ow = np.asarray(o_w, np.float32).astype(BF16)
    qb = np.asarray(q_b, np.float32) / 8.0
    kb = np.asarray(k_b, np.float32)
    vb = np.asarray(v_b, np.float32)
    ob = np.asarray(o_b, np.float32)

    in_maps = []
    for c in range(N_CORES):
        b, r = divmod(c, 4)
        own = slice(OWN * r, OWN * (r + 1))
        m01 = (mask[b] != 0).astype(np.float32)
        in_maps.append(
            {
                "xT": np.ascontiguousarray(input[b].T.astype(BF16)),
                "wq": np.ascontiguousarray(qw[own, :].T),
                "wk": np.ascontiguousarray(kw[own, :].T),
                "wv": np.ascontiguousarray(vw[own, :].T),
                "wo": np.ascontiguousarray(ow[own, :].T),
                "qb2": np.ascontiguousarray(qb[own].reshape(2, 128).T),
                "kb2": np.ascontiguousarray(kb[own].reshape(2, 128).T),
                "vb2": np.ascontiguousarray(vb[own].reshape(2, 128).T),
                "ob2": np.ascontiguousarray(ob[own].reshape(2, 128).T),
                "mask01": np.ascontiguousarray(m01.reshape(KT, 128).T),
            }
        )

    trace = os.environ.get("KERNEL_TRACE", "0") == "1"
    res = run_bass_kernel_spmd(
        nc,
        in_maps,
        core_ids=list(range(N_CORES)),
        trace=trace,
        trace_cores=list(range(N_CORES)) if trace else None,
        stitch_traces=False,
    )
    LAST_EXEC_NS = res.exec_time_ns
    LAST_RESULTS = res

    out = np.empty((BS, L, D), dtype=np.float32)
    for c in range(N_CORES):
        b, r = divmod(c, 4)
        out[b, :, OWN * r : OWN * (r + 1)] = res.results[c]["outT"].T
    return out
